# revision 8
# baseline (speedup 1.0000x reference)
"""AdditiveScorer Trainium2 kernel.

logits[b,q,k] = W2 . tanh(keys[b,k] @ W1[:D] + queries[b,q] @ W1[D:] + b1) + b2
B=2, NQ=NK=1024, D=512, H=32.

Strategy (8 NeuronCores, SPMD):
  - Shard: core c -> batch b = c//4, key-slab ks = c%4 (256 keys). Each core
    computes logits[b, :, ks*256:(ks+1)*256] = [1024, 256].
  - Inputs are passed pre-transposed ([D, n]) in single-pass bf16 (the
    2e-2 rel-err budget makes the fp32-exact hi/lo 3-pass split of the
    earlier version unnecessary): halves input DMA bytes and cuts the
    first-layer matmul count 3x, pulling the first tanh from ~16us to ~6us.
  - W1q is host-replicated 4x so the matmul directly emits the
    kk-replicated hqT layout [128, NQ].
  - Pack partitions as p = kk*32 + h (4 keys x 32 hidden). Per key-group gk
    (4 keys), ONE activation instruction computes
        tanh(hqT_rep[p, q] + bias[p])        # [128, 1024]
    where the per-partition bias carries hk[group gk, kk] + b1[h]: the
    broadcast-add is free via the ACT bias port. The key slab is
    host-permuted so the bias table is four contiguous slices of hkT.
  - Reduction over h applies W2 with the tanh tile as the matmul's
    STATIONARY operand: out[q, kk'] = tanh_tile[:, qchunk].T @ w2sel, with
    w2sel[kk*32+h, j] = W2[h] * (kk == j). Exact fp32. Output lands [q, k]
    in PSUM; finished column slices are drained mid-loop on the idle DVE.
"""

import ml_dtypes
import numpy as np

import concourse.bass as bass
import concourse.tile as tile
from concourse import mybir
from concourse.bass_utils import run_bass_kernel_spmd

F32 = mybir.dt.float32
BF16 = mybir.dt.bfloat16

B, NQ, NK, D, H = 2, 1024, 1024, 512, 32
N_CORES = 8
KSLAB = NK // 4          # keys per core
KGROUPS = KSLAB // 4     # 4-key groups per core
QCHUNKS = NQ // 128      # 128-query chunks
W1W = H + 128            # w1 block: [0:H]=W1k, [H:H+128]=W1q replicated 4x


def _split_multi_waits(nc):
    """The walrus build in this environment rejects any instruction carrying
    more than one sync wait ("Too many sync wait commands"). Hoist all but
    one wait of each instruction onto single-wait NoOp carriers inserted
    just before it in the same engine's stream."""
    for f in nc.m.functions:
        for blk in f.blocks:
            out = []
            changed = False
            for inst in blk.instructions:
                si = inst.sync_info
                waits = list(si.on_wait) if si is not None else []
                if len(waits) > 1:
                    si_cls = type(si)
                    for j, w in enumerate(waits[:-1]):
                        nop = mybir.InstNoOp(name=f"{inst.name}-w{j}", ins=[], outs=[])
                        nop.engine = inst.engine
                        nop.sync_info = si_cls(on_wait=[w], on_update=[])
                        out.append(nop)
                    si.on_wait = [waits[-1]]
                    changed = True
                out.append(inst)
            if changed:
                blk.instructions = out


DRAIN_MODE = "fine"   # "half" | "quarters" | "eighths" | "fine"
ACT_BUFS = 6


def _build_program(n_groups=KGROUPS, debug=False):
    nc = bass.Bass()

    # pblock[:, 0:4] = w2sel; pblock[0:32, 4] = b1; pblock[:, 5] = b2
    pblock = nc.dram_tensor("pblock", [128, 6], F32, kind="ExternalInput")
    w1_d = nc.dram_tensor("w1d", [D, W1W], BF16, kind="ExternalInput")
    q_d = nc.dram_tensor("qd", [D, NQ], BF16, kind="ExternalInput")
    k_d = nc.dram_tensor("kd", [D, KSLAB], BF16, kind="ExternalInput")
    out_l = nc.dram_tensor("out_l", [NQ, KSLAB], F32, kind="ExternalOutput")
    if debug:
        dbg_hq = nc.dram_tensor("dbg_hq", [128, NQ], F32, kind="ExternalOutput")
        dbg_bk = nc.dram_tensor("dbg_bk", [128, KGROUPS], F32, kind="ExternalOutput")

    with tile.TileContext(nc) as tc:
        with (
            tc.tile_pool(name="consts", bufs=1) as consts,
            tc.tile_pool(name="persist", bufs=1) as persist,
        ):
            # ---- parameters ----
            pb = consts.tile([128, 6], F32, tag="pb")
            nc.sync.dma_start(pb[:], pblock[:])
            w2sel_s = pb[:, 0:4]
            b1_s = pb[0:H, 4:5]
            b2_s = pb[:, 5:6]
            w1t = consts.tile([128, D // 128, W1W], BF16, tag="w1t")
            nc.sync.dma_start(w1t[:], w1_d.rearrange("(c p) h -> p c h", p=128))
            w1f = w1t.rearrange("p c h -> p (c h)")

            def w1ap(c, lo, hi):
                return w1f[:, c * W1W + lo:c * W1W + hi]

            # warm the ACT tanh table early
            scratch = consts.tile([128, 1], F32, tag="scratch")
            nc.scalar.activation(scratch[:], pb[:, 5:6],
                                 mybir.ActivationFunctionType.Tanh)

            b_keys = persist.tile([128, KGROUPS], F32, tag="b_keys")

            # main-loop SBUF pools created BEFORE the (released) staging
            # pools so their zones are disjoint: the first tanh then carries
            # no anti-dependency waits on staging-buffer readers.
            act_cm = tc.tile_pool(name="act", bufs=ACT_BUFS)
            outp_cm = tc.tile_pool(name="outp", bufs=1)
            ppers_cm = tc.tile_pool(name="ppers", bufs=1, space="PSUM")
            act_pool = act_cm.__enter__()
            outp = outp_cm.__enter__()
            ppers = ppers_cm.__enter__()
            # hq stays in PSUM for the whole main loop (2 banks); the tanh
            # reads it directly, killing the PSUM->SBUF staging copies.
            ph_all = ppers.tile([128, NQ], F32, tag="ph_all")

            with (
                tc.tile_pool(name="tmats", bufs=1) as tmats,
                tc.tile_pool(name="small", bufs=1) as small,
                tc.tile_pool(name="pmm", bufs=1, space="PSUM") as pmm,
            ):
                # ---- load pre-transposed bf16 inputs ----
                kt = tmats.tile([128, D // 128, KSLAB], BF16, tag="kt")
                nc.sync.dma_start(kt[:], k_d.rearrange("(c p) k -> p c k", p=128))
                ktf = kt.rearrange("p c k -> p (c k)")
                qt = tmats.tile([128, D // 128, NQ], BF16, tag="qt")
                qv = q_d.rearrange("(c p) q -> p c q", p=128)
                # two pieces (kh halves): few DGE slots (HWDGE is a serial
                # global resource at ~625ns each) yet the kh0 matmuls can
                # start while kh1 is still in flight
                for kh in range(2):
                    nc.sync.dma_start(
                        qt[:, :, kh * 512:(kh + 1) * 512],
                        qv[:, :, kh * 512:(kh + 1) * 512])
                qtf = qt.rearrange("p c q -> p (c q)")

                # ---- hkT = W1k.T @ kT + b1 -> bias table (keys first: the
                # chain is short and unblocks the first ACT early) ----
                phk = pmm.tile([H, KSLAB], F32, tag="phk")
                for c in range(4):
                    nc.tensor.matmul(
                        phk[:], w1ap(c, 0, H),
                        ktf[:, c * KSLAB:(c + 1) * KSLAB],
                        start=(c == 0), stop=(c == 3),
                    )
                hkT = small.tile([H, KSLAB], F32, tag="hkT")
                nc.vector.tensor_scalar_add(hkT[:], phk[:], b1_s)
                # key slab host-permuted: position kk*64+gk = original key
                # 4*gk+kk, so the bias table is four contiguous slices
                for kk in range(4):
                    nc.sync.dma_start(
                        b_keys[kk * 32:(kk + 1) * 32, :],
                        hkT[:, kk * KGROUPS:(kk + 1) * KGROUPS])
                # ACT observes the four b_keys DMA sems during its idle
                # window, so the first tanh carries none of those waits
                # (Tile's per-engine vector clock elides observed ticks)
                nc.scalar.copy(scratch[:], b_keys[0:128, 0:1])

                # ---- ph_all = (W1q rep4).T @ qT, single-pass bf16, straight
                # into the persistent PSUM tile (one bank per kh half) ----
                for kh in range(2):
                    for c in range(4):
                        rhs = qtf[:, c * NQ + kh * 512:c * NQ + (kh + 1) * 512]
                        nc.tensor.matmul(
                            ph_all[:, kh * 512:(kh + 1) * 512],
                            w1ap(c, H, W1W), rhs,
                            start=(c == 0), stop=(c == 3),
                        )

            if debug:
                nc.scalar.copy(dbg_scratch := outp.tile([128, NQ], F32, tag="dbgs"), ph_all[:])
                nc.sync.dma_start(dbg_hq[:], dbg_scratch[:])
                nc.sync.dma_start(dbg_bk[:], b_keys[:])

            # ---- main loop ----
            with (
                tc.tile_pool(name="pmain", bufs=1, space="PSUM") as pmain,
            ):
                blk2 = [pmain.tile([128, 2, KSLAB], F32, name=f"blk{j}",
                                   tag=f"blk{j}") for j in range(QCHUNKS // 2)]
                blk = [blk2[qt_ // 2][:, qt_ % 2, :] for qt_ in range(QCHUNKS)]
                o_all = outp.tile([128, QCHUNKS, KSLAB], F32, tag="o_all")
                ov = out_l.rearrange("(qt p) k -> p qt k", p=128)
                if DRAIN_MODE == "half":
                    marks = [n_groups // 2]
                elif DRAIN_MODE == "quarters":
                    marks = [n_groups // 2, 3 * n_groups // 4]
                elif DRAIN_MODE == "eighths":
                    marks = [n_groups * i // 8 for i in range(4, 8)]
                elif DRAIN_MODE == "fine":
                    marks = [24, 36, 46, 54, 58, 61, 63]
                else:
                    marks = [max(1, round(n_groups * f)) for f in DRAIN_MODE]
                drains = {}
                prev = 0
                for m in marks:
                    drains[m - 1] = (4 * prev, 4 * m)
                    prev = m
                final_lo = 4 * prev
                for gk in range(n_groups):
                    act_t = act_pool.tile([128, NQ], F32, tag="act")
                    nc.scalar.activation(
                        act_t[:], ph_all[:],
                        mybir.ActivationFunctionType.Tanh,
                        bias=b_keys[:, gk:gk + 1], scale=1.0,
                    )
                    for qt_ in range(QCHUNKS):
                        nc.tensor.matmul(
                            blk[qt_][:, 4 * gk:4 * gk + 4],
                            act_t[:, qt_ * 128:(qt_ + 1) * 128],
                            w2sel_s,
                            start=True, stop=True,
                        )
                    if gk in drains and n_groups == KGROUPS:
                        # columns [lo:hi) of every psum block are final: drain
                        # on the (idle) DVE while the loop continues
                        lo, hi = drains[gk]
                        for qt_ in range(QCHUNKS):
                            nc.vector.tensor_scalar_add(
                                o_all[:, qt_, lo:hi], blk[qt_][:, lo:hi], b2_s)
                        nc.sync.dma_start(ov[:, :, lo:hi], o_all[:, :, lo:hi])
                # final slice: DVE and ACT in parallel
                lo = final_lo if n_groups == KGROUPS else 0
                for qt_ in range(4):
                    nc.vector.tensor_scalar_add(
                        o_all[:, qt_, lo:KSLAB], blk[qt_][:, lo:KSLAB], b2_s)
                for qt_ in range(4, QCHUNKS):
                    nc.scalar.activation(o_all[:, qt_, lo:KSLAB],
                                         blk[qt_][:, lo:KSLAB],
                                         mybir.ActivationFunctionType.Identity,
                                         bias=b2_s, scale=1.0)
                nc.sync.dma_start(ov[:, :, lo:KSLAB], o_all[:, :, lo:KSLAB])
            ppers_cm.__exit__(None, None, None)
            outp_cm.__exit__(None, None, None)
            act_cm.__exit__(None, None, None)

    _split_multi_waits(nc)
    return nc


_PROGRAM_CACHE = {}


def build_in_maps(keys, queries, W1, b1, W2, b2):
    keys = np.asarray(keys, dtype=np.float32)
    queries = np.asarray(queries, dtype=np.float32)
    W1 = np.asarray(W1, dtype=np.float32)
    b1 = np.asarray(b1, dtype=np.float32)
    W2 = np.asarray(W2, dtype=np.float32)
    b2 = np.asarray(b2, dtype=np.float32)

    pblock = np.zeros((128, 6), dtype=np.float32)
    for j in range(4):
        pblock[j * 32:(j + 1) * 32, j] = W2[:, 0]
    pblock[0:H, 4] = b1
    pblock[:, 5] = float(b2[0])
    w1c = np.hstack([W1[:D], np.tile(W1[D:], (1, 4))])   # [D, W1W]
    w1d = np.ascontiguousarray(w1c.astype(ml_dtypes.bfloat16))

    qT = [np.ascontiguousarray(queries[b].T.astype(ml_dtypes.bfloat16))
          for b in range(B)]                           # [D, NQ] bf16
    kT = np.transpose(keys, (0, 2, 1))                 # [B, D, NK]
    # slab position kk*64+gk holds original slab key 4*gk+kk
    perm = (4 * np.arange(KGROUPS)[None, :] + np.arange(4)[:, None]).ravel()

    in_maps = []
    for c in range(N_CORES):
        b, ks = divmod(c, 4)
        idx = ks * KSLAB + perm
        kd = np.ascontiguousarray(kT[b][:, idx].astype(ml_dtypes.bfloat16))
        in_maps.append({
            "qd": qT[b], "kd": kd,
            "pblock": pblock, "w1d": w1d,
        })
    return in_maps


def kernel(keys, queries, W1, b1, W2, b2):
    if "nc" not in _PROGRAM_CACHE:
        _PROGRAM_CACHE["nc"] = _build_program()
    nc = _PROGRAM_CACHE["nc"]

    in_maps = build_in_maps(keys, queries, W1, b1, W2, b2)
    res = run_bass_kernel_spmd(nc, in_maps, list(range(N_CORES)))

    out = np.empty((B, NQ, NK), dtype=np.float32)
    for c in range(N_CORES):
        b, ks = divmod(c, 4)
        out[b, :, ks * KSLAB:(ks + 1) * KSLAB] = res.results[c]["out_l"]
    return out
